# revision 9
# baseline (speedup 1.0000x reference)
"""Bi-directional correlation cost volume on 8 Trainium2 NeuronCores.

Strategy (data-parallel over batch, one batch element per core):
  - Per core, compute the Gram band G[u, x] = sum_c L[c,h,u] * R[c,h,x] / C
    for |x - u| <= 63 with TensorE matmuls in bf16 (K=C=32, 4x row-tiled
    over h-quadrants via tile_position; quadrant-interleaved issue order).
  - u is tiled in 64-row blocks, each with its own tight 190/127-wide
    x-window; two equal-width blocks share one 128-partition PSUM/staging
    region via col-tiled matmuls (tile_position col offset 64), so copies
    and stores carry no padding.
  - Matmuls pack 2-4 h-rows per PSUM bank; one DVE/ACT copy per bank
    (scale 1/C, cast to bf16) into SBUF staging tiles.
  - Stage the band to HBM as region tensors [P, H, W] (bf16, tight), so
    each store DMA writes per-partition-contiguous runs of G*W*2 bytes.
  - The cost volume out[p, x] = G[x - k(p), x] is a shear of the band; the
    host extracts it with one precomputed vectorized gather per batch.
"""

import numpy as np

B, C, H, WIMG, D = 8, 32, 160, 320, 64
HQ = H // 4   # h-rows per PE quadrant
HGRP = 10     # h-rows per staging tile / store DMA

# Regions: equal-width 64-row u-blocks packed into one partition dim.
# block = (u0, poff, U, xw0); all blocks of a region share width W.
REGIONS = [
    dict(W=127, blocks=[(0, 0, 64, 0), (256, 64, 64, 193)]),
    dict(W=190, blocks=[(64, 0, 64, 1), (128, 64, 64, 65)]),
    dict(W=190, blocks=[(192, 0, 64, 129)]),
]
PTOTS = [sum(b[2] for b in r["blocks"]) for r in REGIONS]
SIZES = [PTOTS[i] * H * REGIONS[i]["W"] for i in range(len(REGIONS))]
ROFFS = [0, SIZES[0], SIZES[0] + SIZES[1]]
WMAX = max(r["W"] for r in REGIONS)
# per 64-wide u-block b: (region, poff, xw0)
BLK = {0: (0, 0, 0), 1: (1, 0, 1), 2: (1, 64, 65), 3: (2, 0, 129),
       4: (0, 64, 193)}

_CACHE = {}


def _bf16():
    from concourse import mybir
    return mybir.dt.np(mybir.dt.bfloat16)


def _get_nc(reps=1):
    key = ("nc", reps, HGRP)
    if key in _CACHE:
        return _CACHE[key]
    import concourse.bacc as bacc
    import concourse.tile as tile
    from concourse import mybir

    f32 = mybir.dt.float32
    bf16 = mybir.dt.bfloat16
    nc = bacc.Bacc("TRN2", target_bir_lowering=False, debug=False)
    r_in = nc.declare_dram_parameter("r_in", [C, H, WIMG], bf16, isOutput=False)
    l_in = nc.declare_dram_parameter("l_in", [C, H, WIMG], bf16, isOutput=False)
    stags = [
        nc.declare_dram_parameter(f"stag{ri}", [PTOTS[ri], H, r["W"]], bf16,
                                  isOutput=True)
        for ri, r in enumerate(REGIONS)
    ]

    with tile.TileContext(nc) as tc:
        with tc.tile_pool(name="inp", bufs=1) as inp_pool, \
             tc.tile_pool(name="ps", bufs=8, space="PSUM") as ps_pool, \
             tc.tile_pool(name="st", bufs=12) as st_pool:
            Lsb = inp_pool.tile([128, HQ * WIMG], bf16, tag="L")
            Rsb = inp_pool.tile([128, HQ * WIMG], bf16, tag="R")
            # partition (q, c) holds h-rows [40q, 40q+40) of channel c
            for q in range(4):
                nc.sync.dma_start(
                    Lsb[32 * q:32 * (q + 1), :],
                    l_in[:, HQ * q:HQ * (q + 1), :].rearrange(
                        "c hh x -> c (hh x)"),
                )
                nc.sync.dma_start(
                    Rsb[32 * q:32 * (q + 1), :],
                    r_in[:, HQ * q:HQ * (q + 1), :].rearrange(
                        "c hh x -> c (hh x)"),
                )
            cctr = 0  # copy-engine round robin
            dctr = 0  # dma-engine round robin
            for _ in range(reps):
                for hh0 in range(0, HQ, HGRP):
                    G = min(HGRP, HQ - hh0)
                    for ri, reg in enumerate(REGIONS):
                        W, blocks = reg["W"], reg["blocks"]
                        PTOT = PTOTS[ri]
                        P = 512 // W  # h-rows packed per PSUM bank
                        sbs = [st_pool.tile([128, HGRP * WMAX], bf16,
                                            tag="sb", name=f"sb{_q}")
                               for _q in range(4)]
                        nbank = (G + P - 1) // P
                        for pi in range(nbank):
                            cnt = min(P, G - pi * P)
                            pss = [ps_pool.tile([128, 512], f32, tag="ps",
                                                name=f"ps{_q}")
                                   for _q in range(4)]
                            # quadrant-interleaved so the 4 row-tiles of
                            # the PE array have concurrent work
                            for s in range(cnt):
                                for q in range(4):
                                    hh = hh0 + pi * P + s
                                    base = hh * WIMG
                                    for (u0, poff, U, xw0) in blocks:
                                        nc.tensor.matmul(
                                            pss[q][poff:poff + U,
                                                   s * W:s * W + W],
                                            Lsb[32 * q:32 * (q + 1),
                                                base + u0:base + u0 + U],
                                            Rsb[32 * q:32 * (q + 1),
                                                base + xw0:base + xw0 + W],
                                            start=True, stop=True,
                                            tile_position=(32 * q, poff),
                                        )
                            for q in range(4):
                                dst = sbs[q][:PTOT,
                                             pi * P * W:(pi * P + cnt) * W]
                                src = pss[q][:PTOT, :cnt * W]
                                if cctr % 2 == 0:
                                    nc.scalar.mul(dst, src, 1.0 / C)
                                else:
                                    nc.vector.tensor_scalar_mul(
                                        dst, src, 1.0 / C)
                                cctr += 1
                        for q in range(4):
                            h0 = HQ * q + hh0
                            dst = stags[ri][:, h0:h0 + G, :]
                            src = sbs[q][:PTOT, :G * W].rearrange(
                                "u (g w) -> u g w", g=G)
                            eng = nc.sync if dctr % 2 else nc.gpsimd
                            dctr += 1
                            eng.dma_start(dst, src)
    nc.compile()
    _CACHE[key] = nc
    return nc


def _gather_idx():
    """IDX [2D, H, W] into the concatenated staging flat; valid mask."""
    if "idx" in _CACHE:
        return _CACHE["idx"]
    p = np.arange(2 * D)[:, None]
    k = np.where(p < D, p, -(p - D))      # signed disparity per plane
    x = np.arange(WIMG)[None, :]
    u = x - k                             # [2D, W]
    valid = (u >= 0) & (u < WIMG)
    uc = np.clip(u, 0, WIMG - 1)
    b = uc // 64                          # 64-wide u-block id
    r = np.choose(b, [BLK[i][0] for i in range(5)])
    poff = np.choose(b, [BLK[i][1] for i in range(5)])
    xw0 = np.choose(b, [BLK[i][2] for i in range(5)])
    Wr = np.choose(r, [reg["W"] for reg in REGIONS])
    off = np.choose(r, ROFFS)
    w = np.clip(x - xw0, 0, Wr - 1)
    base = off + (poff + uc - 64 * b) * (H * Wr) + w    # [2D, W], h=0
    idx = (base[:, None, :]
           + np.arange(H)[None, :, None] * Wr[:, None, :]).astype(np.int32)
    vmask = valid[:, None, :].astype(np.float32)        # [2D, 1, W]
    _CACHE["idx"] = (idx, vmask)
    return _CACHE["idx"]


def _assemble(stag_b):
    """stag_b: tuple of region arrays (bf16) -> out_b [2D, H, W] f32."""
    idx, vmask = _gather_idx()
    flat = np.concatenate([np.asarray(s).ravel() for s in stag_b])
    out = flat[idx].astype(np.float32)
    out *= vmask
    return out


def run_cores(right_np, left_np, timing_reps=0):
    """Run the SPMD bass kernel; returns list of per-batch staging tuples."""
    from concourse.bass_utils import run_bass_kernel_spmd

    bf = _bf16()
    nc = _get_nc()
    in_maps = [
        {"r_in": np.ascontiguousarray(right_np[b]).astype(bf),
         "l_in": np.ascontiguousarray(left_np[b]).astype(bf)}
        for b in range(B)
    ]
    res = run_bass_kernel_spmd(nc, in_maps, list(range(B)))
    return [tuple(res.results[b][f"stag{ri}"] for ri in range(len(REGIONS)))
            for b in range(B)]


def kernel(right_feature, left_feature, max_disp):
    assert int(max_disp) == D
    right_np = np.asarray(right_feature, dtype=np.float32)
    left_np = np.asarray(left_feature, dtype=np.float32)
    stags = run_cores(right_np, left_np)
    out = np.stack([_assemble(s) for s in stags])
    return out
